# revision 14
# baseline (speedup 1.0000x reference)
"""GAT layer kernel for Trainium2 (Bass/Tile), 8-core data-parallel over batch.

Reference (B=16, N=1024, IN_DIM=128, H=4, D=64):
    h = (x @ W).reshape(B,N,H,D)
    e_src/e_dst = einsum('bnhd,hd->bnh', h, a_src/a_dst)
    e[b,i,j,h] = leakyrelu(e_src[b,i,h] + e_dst[b,j,h], 0.2)
    alpha = softmax_j(where(adj[i,j], e, -inf));  out = alpha @ h

Kernel strategy (per core, 2 batches):
  Softmax shift-invariance: with y = s_i + d_j, lrelu(y) = 0.2 s_i + 0.2 d_j
  + 0.8 relu(y); the 0.2 s_i term is constant over j and cancels. So the
  (unnormalized) score reduces to
      PT[j,i] = max(u8_i * V_j, w_j) * m[j,i]
  with u8 = exp(0.8 e_src), V = exp(e_dst), w = exp(0.2 e_dst).

  The N^2*H elementwise score work is split across three engine paths per
  (b, jc) group of 128 j's:
    D: DVE tensor_scalar (mult,max fused, 4x mode) per head + one
       tensor_tensor mask multiply (2x mode) over all 4 heads.
    A: ACT qt = u8*V (activation copy with per-partition column scale) +
       GPSIMD scalar_tensor_tensor (max w, mult mask).
    P: PE rank-1 outer product z = u8_i * V_j into PSUM + GPSIMD
       scalar_tensor_tensor from PSUM (max w, mult mask).
  u8 is broadcast across partitions via a DRAM round-trip DMA (stride-0
  source AP) instead of PE one-hot matmuls + per-chunk ACT exps.
  Row-sums ride separate 1-column matmuls; normalization is a batched
  reciprocal + broadcast tensor_tensor multiply on DVE for both batches.
  All heavy matmuls use bf16 or fp32r (1 PE cycle/row vs 4 for fp32).
"""

import os
import sys
from contextlib import ExitStack

import numpy as np
import ml_dtypes

for _p in ("/opt/trn_rl_repo", "/root/.axon_site/_ro/trn_rl_repo"):
    if os.path.isdir(_p) and _p not in sys.path:
        sys.path.insert(0, _p)

import concourse.bass as bass
import concourse.mybir as mybir
import concourse.tile as tile

F32 = mybir.dt.float32
F32R = mybir.dt.float32r
BF16 = mybir.dt.bfloat16
AF = mybir.ActivationFunctionType
ALU = mybir.AluOpType
NPBF = ml_dtypes.bfloat16

B, N, IN_DIM, H, D = 16, 1024, 128, 4, 64
HD = H * D            # 256
NCORES = 8
BL = B // NCORES      # 2 batches per core
NTC = N // 128        # 8 chunks of 128

# score-group engine assignment: (b, jc) -> 'D' | 'V'
#   D: DVE tensor_scalar (mult,max) + DVE tensor_tensor mask
#   V: DVE tensor_scalar (mult,max) + GPSIMD tensor_tensor mask
# (GPSIMD supports only plain tensor_tensor/tensor_scalar here: the fused
#  STT struct and PSUM operands are rejected by the walrus backend.)
MODE = {}
for _b in range(BL):
    for _jc in range(NTC):
        MODE[(_b, _jc)] = "D"
for _g in [(0, 1), (0, 4), (0, 6), (1, 1), (1, 4), (1, 6)]:
    MODE[_g] = "V"


def _split_excess_waits(nc, max_waits=1):
    """Walrus codegen rejects compute instructions carrying more than one
    sync wait. Move the extras onto engine-matched NoOps inserted
    immediately before the instruction."""
    def _steal_nop(engine):
        engine.nop()
        for fn in nc.m.functions:
            for blk in fn.blocks:
                il = blk.instructions
                if il and type(il[-1]).__name__ == "InstNoOp":
                    nop = il[-1]
                    blk.instructions = il[:-1]
                    return nop
        raise RuntimeError("could not locate appended nop")

    for fn in nc.m.functions:
        for blk in fn.blocks:
            il = list(blk.instructions)
            out = []
            changed = False
            for inst in il:
                si = inst.sync_info
                if (type(inst).__name__ != "InstNoOp" and si is not None
                        and len(si.on_wait) > max_waits):
                    waits = list(si.on_wait)
                    for w in waits[max_waits:]:
                        nop = _steal_nop(nc.engines[inst.engine])
                        nop.sync_info = mybir.SyncInfo(on_wait=[w], on_update=[])
                        out.append(nop)
                    inst.sync_info = mybir.SyncInfo(
                        on_wait=waits[:max_waits], on_update=list(si.on_update))
                    changed = True
                out.append(inst)
            if changed:
                blk.instructions = out
    return nc


def build_gat_program():
    nc = bass.Bass("TRN2", target_bir_lowering=False, debug=False)
    xT_d = nc.dram_tensor("xT", (BL, IN_DIM, N), F32R, kind="ExternalInput").ap()
    W_d = nc.dram_tensor("W", (IN_DIM, HD), F32R, kind="ExternalInput").ap()
    WAcat_d = nc.dram_tensor("WAcat", (IN_DIM, 36), F32R, kind="ExternalInput").ap()
    maskT_d = nc.dram_tensor("maskT", (N, N), BF16, kind="ExternalInput").ap()
    u8st_d = nc.dram_tensor("u8st", (BL, H, N), BF16, kind="Internal").ap()
    out_d = nc.dram_tensor("out", (BL, N, HD), BF16, kind="ExternalOutput").ap()

    with tile.TileContext(nc) as tc:
        with ExitStack() as ctx:
            _gat_body(ctx, tc, out_d, xT_d, W_d, WAcat_d, maskT_d, u8st_d)
    _split_excess_waits(nc)
    return nc


def _gat_body(ctx, tc, out_d, xT_d, W_d, WAcat_d, maskT_d, u8st_d):
    nc = tc.nc

    consts = ctx.enter_context(tc.tile_pool(name="consts", bufs=1))
    persist = ctx.enter_context(tc.tile_pool(name="persist", bufs=1))
    qt_pool = ctx.enter_context(tc.tile_pool(name="qt", bufs=4))
    qwm_pool = ctx.enter_context(tc.tile_pool(name="qwm", bufs=6))
    osb_pool = ctx.enter_context(tc.tile_pool(name="osb", bufs=3))
    rcl_pool = ctx.enter_context(tc.tile_pool(name="rcl", bufs=3))
    ps_z = ctx.enter_context(tc.tile_pool(name="ps_z", bufs=2, space="PSUM"))
    ps_p1 = ctx.enter_context(tc.tile_pool(name="ps_p1", bufs=1, space="PSUM"))
    ps_acc = ctx.enter_context(tc.tile_pool(name="ps_acc", bufs=1, space="PSUM"))

    # ---- constants / inputs resident in SBUF ----
    # xT b0 first: its descriptor-gen overlaps the tiny weight transfers
    xT_sb = consts.tile([128, BL, N], F32R)
    nc.sync.dma_start(out=xT_sb[:, 0, 0:512], in_=xT_d[0][:, 0:512])
    nc.sync.dma_start(out=xT_sb[:, 0, 512:], in_=xT_d[0][:, 512:])
    WAcat_sb = consts.tile([128, 36], F32R)
    nc.sync.dma_start(out=WAcat_sb, in_=WAcat_d)
    nc.sync.dma_start(out=xT_sb[:, 1, :], in_=xT_d[1])
    W_sb = consts.tile([128, HD], F32R)
    nc.sync.dma_start(out=W_sb, in_=W_d)
    ones_col = consts.tile([128, 1], BF16)
    nc.vector.memset(ones_col, 1.0)
    maskT_sb = consts.tile([128, NTC, N], BF16)
    nc.sync.dma_start(
        out=maskT_sb,
        in_=maskT_d.rearrange("(jc p) i -> p jc i", p=128))

    # ---- persistent per-batch intermediates ----
    haug_sb = persist.tile([128, BL, NTC, HD], BF16)   # [j-in-chunk, b, jc, h*64+d]
    u8row_sb = persist.tile([4, BL, N], BF16)          # exp(0.8 e_src) rows
    Vcol_sb = persist.tile([128, BL, NTC, H], F32)     # exp(e_dst) cols
    wcol_sb = persist.tile([128, BL, NTC, H], F32)     # exp(0.2 e_dst) cols
    U8bc = persist.tile([128, BL, H, N], BF16)         # u8 broadcast over parts

    # ---- phase 1: E = x @ WAcat (rows + cols), haug = x @ W ----
    for b in range(BL):
        # E rows [a=src4+dst4, t] via two 512-col halves (z-pool slots);
        # exp straight out of PSUM into the u8/V row tiles
        for half in range(2):
            e8 = ps_z.tile([128, 512], F32, tag="z", name=f"e8_{b}_{half}")
            nc.tensor.matmul(e8[0:36, :], lhsT=WAcat_sb,
                             rhs=xT_sb[:, b, half * 512:(half + 1) * 512],
                             start=True, stop=True)
            sl = slice(half * 512, (half + 1) * 512)
            nc.scalar.activation(u8row_sb[:, b, sl], e8[0:4, :], AF.Exp,
                                 bias=0.0, scale=0.8)
        # u8 rows -> DRAM, then broadcast-read across all 128 partitions
        nc.sync.dma_start(out=u8st_d[b], in_=u8row_sb[:, b, :])
        nc.sync.dma_start(
            out=U8bc[:, b],
            in_=u8st_d[b].unsqueeze(0).broadcast_to((128, H, N)))
        # E cols [t, a] per 128-chunk; exp into V / w columns
        ecol_slot = ps_z.tile([128, 512], F32, tag="z", name=f"ecol_{b}")
        ecol = ecol_slot[:, 0:NTC * 36]
        for tcc in range(NTC):
            nc.tensor.matmul(ecol[:, tcc * 36:(tcc + 1) * 36],
                             lhsT=xT_sb[:, b, tcc * 128:(tcc + 1) * 128],
                             rhs=WAcat_sb, start=True, stop=True)
        dstv = ecol.rearrange("p (t a) -> p t a", t=NTC)[:, :, 32:36]
        nc.scalar.activation(Vcol_sb[:, b], dstv, AF.Exp, bias=0.0, scale=1.0)
        nc.scalar.activation(wcol_sb[:, b], dstv, AF.Exp, bias=0.0, scale=0.2)
        # haug[t, h*64+d] = h in bf16 for the alpha@h contraction
        for tcc in range(NTC):
            hp = ps_p1.tile([128, HD], F32, tag="haug")
            nc.tensor.matmul(hp, lhsT=xT_sb[:, b, tcc * 128:(tcc + 1) * 128],
                             rhs=W_sb, start=True, stop=True)
            nc.scalar.activation(haug_sb[:, b, tcc, :], hp, AF.Copy,
                                 bias=0.0, scale=1.0)

    # ---- phase 2: scores + alpha @ h ----
    for b in range(BL):
        # 4 oacc banks hold the 32 (ic,h) 64-col chains; rs holds row-sums
        obank = [ps_acc.tile([128, 512], F32, tag=f"oacc{k}", name=f"oacc{k}_{b}")
                 for k in range(4)]
        rs = ps_acc.tile([128, 32], F32, tag="rs")
        for jc in range(NTC):
            mode = MODE[(b, jc)]
            qwm = qwm_pool.tile([128, H, N], BF16, tag="qwm")
            qt = qt_pool.tile([128, H, N], BF16, tag="qt")
            for h in range(H):
                nc.vector.tensor_scalar(
                    out=qt[:, h, :], in0=U8bc[:, b, h, :],
                    scalar1=Vcol_sb[:, b, jc, h:h + 1],
                    scalar2=wcol_sb[:, b, jc, h:h + 1],
                    op0=ALU.mult, op1=ALU.max)
            mask_eng = nc.gpsimd if mode == "V" else nc.vector
            mask_eng.tensor_tensor(
                out=qwm, in0=qt,
                in1=maskT_sb[:, jc, :].unsqueeze(1).broadcast_to((128, H, N)),
                op=ALU.mult)
            # start=True zeroes a whole 2KB psum bank: only the first chain in
            # each bank starts the group, only the last one stops it. In the
            # final group the row-sum matmuls go first so the reciprocal can
            # start while the data chains finish.
            passes = ([("rs",), ("data",)] if jc == NTC - 1
                      else [("data", "rs")])
            for kinds in passes:
                for h in range(H):
                    for ic in range(NTC):
                        c = ic * 4 + h
                        lhsT = qwm[:, h, ic * 128:(ic + 1) * 128]
                        if "data" in kinds:
                            nc.tensor.matmul(
                                obank[c // 8][:, (c % 8) * 64:(c % 8 + 1) * 64],
                                lhsT=lhsT,
                                rhs=haug_sb[:, b, jc, h * 64:(h + 1) * 64],
                                start=(jc == 0 and c % 8 == 0),
                                stop=(jc == NTC - 1 and c % 8 == 7))
                        if "rs" in kinds:
                            nc.tensor.matmul(rs[:, c:c + 1], lhsT=lhsT,
                                             rhs=ones_col,
                                             start=(jc == 0 and c == 0),
                                             stop=(jc == NTC - 1 and c == 31))
        rcl = rcl_pool.tile([128, 32], F32, tag="rcl")
        nc.vector.reciprocal(rcl, rs)
        osb = osb_pool.tile([128, NTC, HD], BF16, tag="osb")
        half_out = NTC // 2
        for ic in range(NTC):
            oslice = obank[ic // 2][:, (ic % 2) * 256:(ic % 2 + 1) * 256]
            nc.vector.tensor_tensor(
                out=osb[:, ic, :].rearrange("p (h d) -> p h d", h=H),
                in0=oslice.rearrange("p (h d) -> p h d", h=H),
                in1=rcl[:, ic * 4:(ic + 1) * 4].unsqueeze(2)
                    .broadcast_to((128, 4, D)), op=ALU.mult)
            if ic == half_out - 1:
                nc.sync.dma_start(
                    out=out_d[b, 0:half_out * 128].rearrange(
                        "(ic p) d -> p ic d", p=128),
                    in_=osb[:, 0:half_out, :])
        nc.sync.dma_start(
            out=out_d[b, half_out * 128:].rearrange("(ic p) d -> p ic d", p=128),
            in_=osb[:, half_out:, :])


def prep_inputs(x, adj, W, a_src, a_dst):
    """Host-side prep: shard x over cores, build combined weight layouts."""
    x = np.asarray(x, np.float32)
    adj = np.asarray(adj)
    W = np.asarray(W, np.float32)
    a_src = np.asarray(a_src, np.float32)
    a_dst = np.asarray(a_dst, np.float32)

    maskT = np.ascontiguousarray(adj.T).astype(NPBF)
    Acat = np.zeros((HD, 36), np.float32)
    for h in range(H):
        Acat[h * D:(h + 1) * D, h] = a_src[h]
        Acat[h * D:(h + 1) * D, 32 + h] = a_dst[h]
    WAcat = np.ascontiguousarray(W @ Acat)  # (IN_DIM, 36): src at 0-3, dst at 32-35

    in_maps = []
    for c in range(NCORES):
        xT = np.ascontiguousarray(x[c * BL:(c + 1) * BL].transpose(0, 2, 1))
        in_maps.append({"xT": xT, "W": W, "WAcat": WAcat, "maskT": maskT})
    return in_maps


_PROGRAM_CACHE = {}


def _get_program():
    if "nc" not in _PROGRAM_CACHE:
        _PROGRAM_CACHE["nc"] = build_gat_program()
    return _PROGRAM_CACHE["nc"]


def run_on_hw(inputs, trace=False):
    from concourse.bass_utils import run_bass_kernel_spmd
    nc = _get_program()
    in_maps = prep_inputs(**inputs)
    res = run_bass_kernel_spmd(nc, in_maps, list(range(NCORES)), trace=trace)
    out = np.concatenate(
        [np.asarray(res.results[c]["out"]).astype(np.float32)
         for c in range(NCORES)], axis=0)
    return out, res


def kernel(**inputs) -> np.ndarray:
    out, _ = run_on_hw(inputs, trace=False)
    return out


# revision 18
# speedup vs baseline: 1.1646x; 1.1646x over previous
"""GAT layer kernel for Trainium2 (Bass/Tile), 8-core data-parallel over batch.

Reference (B=16, N=1024, IN_DIM=128, H=4, D=64):
    h = (x @ W).reshape(B,N,H,D)
    e_src/e_dst = einsum('bnhd,hd->bnh', h, a_src/a_dst)
    e[b,i,j,h] = leakyrelu(e_src[b,i,h] + e_dst[b,j,h], 0.2)
    alpha = softmax_j(where(adj[i,j], e, -inf));  out = alpha @ h

Kernel strategy (per core, 2 batches):
  Softmax shift-invariance: with y = s_i + d_j, lrelu(y) = 0.2 s_i + 0.2 d_j
  + 0.8 relu(y); the 0.2 s_i term is constant over j and cancels. So the
  (unnormalized) score reduces to
      PT[j,i] = max(u8_i * V_j, w_j) * m[j,i]
  with u8 = exp(0.8 e_src), V = exp(e_dst), w = exp(0.2 e_dst).

  The N^2*H elementwise score work is split across three engine paths per
  (b, jc) group of 128 j's:
    D: DVE tensor_scalar (mult,max fused, 4x mode) per head + one
       tensor_tensor mask multiply (2x mode) over all 4 heads.
    A: ACT qt = u8*V (activation copy with per-partition column scale) +
       GPSIMD scalar_tensor_tensor (max w, mult mask).
    P: PE rank-1 outer product z = u8_i * V_j into PSUM + GPSIMD
       scalar_tensor_tensor from PSUM (max w, mult mask).
  u8 is broadcast across partitions via a DRAM round-trip DMA (stride-0
  source AP) instead of PE one-hot matmuls + per-chunk ACT exps.
  Row-sums ride separate 1-column matmuls; normalization is a batched
  reciprocal + broadcast tensor_tensor multiply on DVE for both batches.
  All heavy matmuls use bf16 or fp32r (1 PE cycle/row vs 4 for fp32).
"""

import os
import sys
from contextlib import ExitStack

import numpy as np
import ml_dtypes

for _p in ("/opt/trn_rl_repo", "/root/.axon_site/_ro/trn_rl_repo"):
    if os.path.isdir(_p) and _p not in sys.path:
        sys.path.insert(0, _p)

import concourse.bass as bass
import concourse.mybir as mybir
import concourse.tile as tile

F32 = mybir.dt.float32
F32R = mybir.dt.float32r
BF16 = mybir.dt.bfloat16
AF = mybir.ActivationFunctionType
ALU = mybir.AluOpType
NPBF = ml_dtypes.bfloat16

B, N, IN_DIM, H, D = 16, 1024, 128, 4, 64
HD = H * D            # 256
NCORES = 8
BL = B // NCORES      # 2 batches per core
NTC = N // 128        # 8 chunks of 128

# score-group engine assignment: (b, jc) -> 'D' | 'V'
#   D: DVE tensor_scalar (mult,max) + DVE tensor_tensor mask
#   V: DVE tensor_scalar (mult,max) + GPSIMD tensor_tensor mask
# (GPSIMD supports only plain tensor_tensor/tensor_scalar here: the fused
#  STT struct and PSUM operands are rejected by the walrus backend.)
MODE = {}
for _b in range(BL):
    for _jc in range(NTC):
        MODE[(_b, _jc)] = "D"
for _g in [(0, 1), (0, 3), (0, 5), (1, 1), (1, 4)]:
    MODE[_g] = "V"


def _split_excess_waits(nc, max_waits=1):
    """Walrus codegen rejects compute instructions carrying more than one
    sync wait. Move the extras onto engine-matched NoOps inserted
    immediately before the instruction."""
    def _steal_nop(engine):
        engine.nop()
        for fn in nc.m.functions:
            for blk in fn.blocks:
                il = blk.instructions
                if il and type(il[-1]).__name__ == "InstNoOp":
                    nop = il[-1]
                    blk.instructions = il[:-1]
                    return nop
        raise RuntimeError("could not locate appended nop")

    for fn in nc.m.functions:
        for blk in fn.blocks:
            il = list(blk.instructions)
            out = []
            changed = False
            for inst in il:
                si = inst.sync_info
                if (type(inst).__name__ != "InstNoOp" and si is not None
                        and len(si.on_wait) > max_waits):
                    waits = list(si.on_wait)
                    for w in waits[max_waits:]:
                        nop = _steal_nop(nc.engines[inst.engine])
                        nop.sync_info = mybir.SyncInfo(on_wait=[w], on_update=[])
                        out.append(nop)
                    inst.sync_info = mybir.SyncInfo(
                        on_wait=waits[:max_waits], on_update=list(si.on_update))
                    changed = True
                out.append(inst)
            if changed:
                blk.instructions = out
    return nc


def build_gat_program():
    nc = bass.Bass("TRN2", target_bir_lowering=False, debug=False)
    xT_d = nc.dram_tensor("xT", (BL, IN_DIM, N), F32R, kind="ExternalInput").ap()
    W_d = nc.dram_tensor("W", (IN_DIM, HD), F32R, kind="ExternalInput").ap()
    WAcat_d = nc.dram_tensor("WAcat", (IN_DIM, 36), F32R, kind="ExternalInput").ap()
    maskT_d = nc.dram_tensor("maskT", (N, N), BF16, kind="ExternalInput").ap()
    u8st_d = nc.dram_tensor("u8st", (BL, H, N), BF16, kind="Internal").ap()
    out_d = nc.dram_tensor("out", (BL, N, HD), BF16, kind="ExternalOutput").ap()

    with tile.TileContext(nc) as tc:
        with ExitStack() as ctx:
            _gat_body(ctx, tc, out_d, xT_d, W_d, WAcat_d, maskT_d, u8st_d)
    _split_excess_waits(nc)
    return nc


def _gat_body(ctx, tc, out_d, xT_d, W_d, WAcat_d, maskT_d, u8st_d):
    nc = tc.nc

    consts = ctx.enter_context(tc.tile_pool(name="consts", bufs=1))
    persist = ctx.enter_context(tc.tile_pool(name="persist", bufs=1))
    qt_pool = ctx.enter_context(tc.tile_pool(name="qt", bufs=4))
    qwm_pool = ctx.enter_context(tc.tile_pool(name="qwm", bufs=6))
    osb_pool = ctx.enter_context(tc.tile_pool(name="osb", bufs=3))
    rcl_pool = ctx.enter_context(tc.tile_pool(name="rcl", bufs=3))
    ps_z = ctx.enter_context(tc.tile_pool(name="ps_z", bufs=2, space="PSUM"))
    ps_p1 = ctx.enter_context(tc.tile_pool(name="ps_p1", bufs=1, space="PSUM"))
    ps_acc = ctx.enter_context(tc.tile_pool(name="ps_acc", bufs=1, space="PSUM"))

    # ---- constants / inputs resident in SBUF ----
    # DMA transfers are serviced serially; order them so b0's score inputs
    # (u8 broadcast, first mask chunks) land as early as possible. The mask
    # load is split per-jc chunk and interleaved below.
    xT_sb = consts.tile([128, BL, N], F32R)
    nc.sync.dma_start(out=xT_sb[:, 0, 0:512], in_=xT_d[0][:, 0:512])
    nc.sync.dma_start(out=xT_sb[:, 0, 512:], in_=xT_d[0][:, 512:])
    WAcat_sb = consts.tile([128, 36], F32R)
    nc.sync.dma_start(out=WAcat_sb, in_=WAcat_d)
    W_sb = consts.tile([128, HD], F32R)
    ones_col = consts.tile([128, 1], BF16)
    nc.vector.memset(ones_col, 1.0)
    maskT_sb = consts.tile([128, NTC, N], BF16)
    maskT_src = maskT_d.rearrange("(jc p) i -> p jc i", p=128)

    def load_mask(jc):
        nc.sync.dma_start(out=maskT_sb[:, jc, :], in_=maskT_src[:, jc, :])

    load_mask(0)
    load_mask(1)

    # ---- persistent per-batch intermediates ----
    haug_sb = persist.tile([128, BL, NTC, HD], BF16)   # [j-in-chunk, b, jc, h*64+d]
    u8row_sb = persist.tile([4, BL, N], BF16)          # exp(0.8 e_src) rows
    Vcol_sb = persist.tile([128, BL, NTC, H], F32)     # exp(e_dst) cols
    wcol_sb = persist.tile([128, BL, NTC, H], F32)     # exp(0.2 e_dst) cols
    U8bc = persist.tile([128, BL, H, N], BF16)         # u8 broadcast over parts

    # ---- phase 1: E = x @ WAcat (rows + cols), haug = x @ W ----
    for b in range(BL):
        # E rows [a=src4+dst4, t] via two 512-col halves (z-pool slots);
        # exp straight out of PSUM into the u8 row tile
        for half in range(2):
            e8 = ps_z.tile([128, 512], F32, tag="z", name=f"e8_{b}_{half}")
            nc.tensor.matmul(e8[0:36, :], lhsT=WAcat_sb,
                             rhs=xT_sb[:, b, half * 512:(half + 1) * 512],
                             start=True, stop=True)
            sl = slice(half * 512, (half + 1) * 512)
            nc.scalar.activation(u8row_sb[:, b, sl], e8[0:4, :], AF.Exp,
                                 bias=0.0, scale=0.8)
        # u8 rows -> DRAM, then broadcast-read across all 128 partitions.
        # Issued from the ACT queue so their data-dependency waits don't
        # block the SP bulk-load queue.
        nc.scalar.dma_start(out=u8st_d[b], in_=u8row_sb[:, b, :])
        nc.scalar.dma_start(
            out=U8bc[:, b],
            in_=u8st_d[b].unsqueeze(0).broadcast_to((128, H, N)))
        if b == 0:
            load_mask(2)
            load_mask(3)
            # xT b1 + W arrive behind b0's critical transfers
            nc.sync.dma_start(out=xT_sb[:, 1, :], in_=xT_d[1])
            nc.sync.dma_start(out=W_sb, in_=W_d)
        else:
            for jc in range(4, NTC):
                load_mask(jc)
        # E cols [t, a] per 128-chunk; exp into V / w columns
        ecol_slot = ps_z.tile([128, 512], F32, tag="z", name=f"ecol_{b}")
        ecol = ecol_slot[:, 0:NTC * 36]
        for tcc in range(NTC):
            nc.tensor.matmul(ecol[:, tcc * 36:(tcc + 1) * 36],
                             lhsT=xT_sb[:, b, tcc * 128:(tcc + 1) * 128],
                             rhs=WAcat_sb, start=True, stop=True)
        dstv = ecol.rearrange("p (t a) -> p t a", t=NTC)[:, :, 32:36]
        nc.scalar.activation(Vcol_sb[:, b], dstv, AF.Exp, bias=0.0, scale=1.0)
        nc.scalar.activation(wcol_sb[:, b], dstv, AF.Exp, bias=0.0, scale=0.2)
        # haug[t, h*64+d] = h in bf16 for the alpha@h contraction
        for tcc in range(NTC):
            hp = ps_p1.tile([128, HD], F32, tag="haug")
            nc.tensor.matmul(hp, lhsT=xT_sb[:, b, tcc * 128:(tcc + 1) * 128],
                             rhs=W_sb, start=True, stop=True)
            nc.scalar.activation(haug_sb[:, b, tcc, :], hp, AF.Copy,
                                 bias=0.0, scale=1.0)

    # ---- phase 2: scores + alpha @ h ----
    for b in range(BL):
        # 4 oacc banks hold the 32 (ic,h) 64-col chains; rs holds row-sums
        obank = [ps_acc.tile([128, 512], F32, tag=f"oacc{k}", name=f"oacc{k}_{b}")
                 for k in range(4)]
        rs = ps_acc.tile([128, 32], F32, tag="rs")
        for jc in range(NTC):
            mode = MODE[(b, jc)]
            qwm = qwm_pool.tile([128, H, N], BF16, tag="qwm")
            qt = qt_pool.tile([128, H, N], BF16, tag="qt")
            for h in range(H):
                nc.vector.tensor_scalar(
                    out=qt[:, h, :], in0=U8bc[:, b, h, :],
                    scalar1=Vcol_sb[:, b, jc, h:h + 1],
                    scalar2=wcol_sb[:, b, jc, h:h + 1],
                    op0=ALU.mult, op1=ALU.max)
            if mode == "V":
                # per-head GPSIMD mask multiplies pipeline better than one
                # big op against the downstream matmuls
                for h in range(H):
                    nc.gpsimd.tensor_tensor(
                        out=qwm[:, h, :], in0=qt[:, h, :],
                        in1=maskT_sb[:, jc, :], op=ALU.mult)
            else:
                nc.vector.tensor_tensor(
                    out=qwm, in0=qt,
                    in1=maskT_sb[:, jc, :].unsqueeze(1).broadcast_to((128, H, N)),
                    op=ALU.mult)
            # start=True zeroes a whole 2KB psum bank: only the first chain in
            # each bank starts the group, only the last one stops it. In the
            # final group the row-sum matmuls go first so the reciprocal can
            # start while the data chains finish.
            passes = ([("rs",), ("data",)] if jc == NTC - 1
                      else [("data", "rs")])
            for kinds in passes:
                for h in range(H):
                    for ic in range(NTC):
                        c = ic * 4 + h
                        lhsT = qwm[:, h, ic * 128:(ic + 1) * 128]
                        if "data" in kinds:
                            nc.tensor.matmul(
                                obank[c // 8][:, (c % 8) * 64:(c % 8 + 1) * 64],
                                lhsT=lhsT,
                                rhs=haug_sb[:, b, jc, h * 64:(h + 1) * 64],
                                start=(jc == 0 and c % 8 == 0),
                                stop=(jc == NTC - 1 and c % 8 == 7))
                        if "rs" in kinds:
                            nc.tensor.matmul(rs[:, c:c + 1], lhsT=lhsT,
                                             rhs=ones_col,
                                             start=(jc == 0 and c == 0),
                                             stop=(jc == NTC - 1 and c == 31))
        rcl = rcl_pool.tile([128, 32], F32, tag="rcl")
        nc.vector.reciprocal(rcl, rs)
        osb = osb_pool.tile([128, NTC, HD], BF16, tag="osb")
        half_out = NTC // 2
        for ic in range(NTC):
            oslice = obank[ic // 2][:, (ic % 2) * 256:(ic % 2 + 1) * 256]
            nc.vector.tensor_tensor(
                out=osb[:, ic, :].rearrange("p (h d) -> p h d", h=H),
                in0=oslice.rearrange("p (h d) -> p h d", h=H),
                in1=rcl[:, ic * 4:(ic + 1) * 4].unsqueeze(2)
                    .broadcast_to((128, 4, D)), op=ALU.mult)
            if ic == half_out - 1:
                nc.sync.dma_start(
                    out=out_d[b, 0:half_out * 128].rearrange(
                        "(ic p) d -> p ic d", p=128),
                    in_=osb[:, 0:half_out, :])
        nc.sync.dma_start(
            out=out_d[b, half_out * 128:].rearrange("(ic p) d -> p ic d", p=128),
            in_=osb[:, half_out:, :])


def prep_inputs(x, adj, W, a_src, a_dst):
    """Host-side prep: shard x over cores, build combined weight layouts."""
    x = np.asarray(x, np.float32)
    adj = np.asarray(adj)
    W = np.asarray(W, np.float32)
    a_src = np.asarray(a_src, np.float32)
    a_dst = np.asarray(a_dst, np.float32)

    maskT = np.ascontiguousarray(adj.T).astype(NPBF)
    Acat = np.zeros((HD, 36), np.float32)
    for h in range(H):
        Acat[h * D:(h + 1) * D, h] = a_src[h]
        Acat[h * D:(h + 1) * D, 32 + h] = a_dst[h]
    WAcat = np.ascontiguousarray(W @ Acat)  # (IN_DIM, 36): src at 0-3, dst at 32-35

    in_maps = []
    for c in range(NCORES):
        xT = np.ascontiguousarray(x[c * BL:(c + 1) * BL].transpose(0, 2, 1))
        in_maps.append({"xT": xT, "W": W, "WAcat": WAcat, "maskT": maskT})
    return in_maps


_PROGRAM_CACHE = {}


def _get_program():
    if "nc" not in _PROGRAM_CACHE:
        _PROGRAM_CACHE["nc"] = build_gat_program()
    return _PROGRAM_CACHE["nc"]


def run_on_hw(inputs, trace=False):
    from concourse.bass_utils import run_bass_kernel_spmd
    nc = _get_program()
    in_maps = prep_inputs(**inputs)
    res = run_bass_kernel_spmd(nc, in_maps, list(range(NCORES)), trace=trace)
    out = np.concatenate(
        [np.asarray(res.results[c]["out"]).astype(np.float32)
         for c in range(NCORES)], axis=0)
    return out, res


def kernel(**inputs) -> np.ndarray:
    out, _ = run_on_hw(inputs, trace=False)
    return out
